# revision 1
# baseline (speedup 1.0000x reference)
"""Trainium2 Bass kernel for DynamicLocalGlobalRouter.

Reference computation (B=2, H=16, S=2048, D=64, radius=16):
  local_out = sliding-window softmax attention (window 33) per (b,h)
  gate      = sigmoid(mean_s(q) @ w_gate + b_gate)      per (b,h)
  out       = gate * local_out + (1-gate) * global_out

Sharding: B*H = 32 (b,h) pairs -> 4 pairs per core across 8 cores.

Device algorithm (per pair), key-stationary banded attention:
  - Host supplies q/k transposed+padded to [64, S+pad] (bf16) so the
    contraction dim (d=64) is on partitions; no on-device transposes.
    Two pairs stack on the partition axis so DMAs use all 16 ports, and
    v||ones and global are merged into one bf16 array per pair.
  - For each key chunk c (17 chunks of 128 keys, shifted by -16):
      scores_T[k, q] = K_T_chunk.T @ Q_T_span        (PE, psum [128,160])
      P_T = exp(scores_T / 8)                        (ACT, batched 6 chunks
                                                      per op; no max-subtract:
                                                      scores are O(1) randn)
      P_T *= band_mask                               (DVE/GpSimd split,
                                                      zeroes out-of-band)
      pv[block] += P_T_cols.T @ [V | 1/g]            (PE; extra column gives
                                                      Z/g = sum of weights)
  - tail: out_blk = pv[:,0:64]*(g/Z) + (1-g)*global_blk, alternating per
    block between one fused DVE scalar_tensor_tensor and ACT-scale+GpSimd-add
The softmax normalization (1/Z), gate, and boundary masking are exact:
zero-padded K columns give exp(0)=1 which the band mask multiplies to 0,
and out-of-range V rows/ones-column entries are zero. bf16 is used for
matmul inputs only; scores, softmax, accumulation and output stay fp32.
"""

import os
import sys
from contextlib import ExitStack

import numpy as np

sys.path.insert(0, "/opt/trn_rl_repo")

import concourse.bacc as bacc  # noqa: E402
import concourse.tile as tile  # noqa: E402
from concourse import mybir  # noqa: E402
from concourse.bass_utils import run_bass_kernel_spmd  # noqa: E402

B, H, S, D = 2, 16, 2048, 64
RADIUS = 16
NCORES = 8
PAIRS = B * H            # 32
PPC = PAIRS // NCORES    # 4 pairs per core
NB = S // 128            # 16 query/key blocks
NCH = NB + 1             # 17 key chunks (chunk c covers keys [c*128-16, c*128+112))
SPAN = 160               # query span per key chunk
QT_W = 32 + S + 128      # 2208 padded Q_T width (col i <-> query i-32)
KT_W = 16 + S + 112      # 2176 padded K_T width (col j <-> key j-16)
VS_W = NCH * 65          # V chunks with appended ones/invg column
GRP = 6                  # score chunks batched per psum group tile
GRP_W = 1024             # group tile width: 2 psum banks, 3 x 160 slots per bank

F32 = mybir.dt.float32
BF16 = mybir.dt.bfloat16

import ml_dtypes  # noqa: E402

NP_BF16 = ml_dtypes.bfloat16

# set by test harness to capture an NTFF profile
TRACE = bool(int(os.environ.get("KERNEL_TRACE", "0")))
LAST_RESULT = None

_CACHE = {}

# bisection knobs for benchmarking: subset of
# {"scores", "exp", "mask", "pv", "tail", "dmain", "gate"}
DISABLE = frozenset()
# bench-only: how many copies of the body to emit per loop iteration
BODY_MULT = 1
# emit group g-1's PV matmuls after group g's scores (software pipelining)
SWPIPE = False


def _goff(l):
    """free-dim offset of chunk-slot l (0..5) inside a group tile; slots
    avoid straddling the 2KB psum bank boundary (3 x 160 <= 512 per bank)"""
    return (l // 3) * 512 + (l % 3) * 160


def _build_program(nc, reps=None):
    # qt/kt stack two pairs on the partition axis (pair 2j on partitions
    # 0:64, pair 2j+1 on 64:128) so their DMAs use all 16 ports.
    qk_d = nc.dram_tensor("qk", [PPC // 2, 128, QT_W + KT_W], BF16, kind="ExternalInput")
    vg_d = nc.dram_tensor("vg", [PPC, 128, VS_W + NB * 64], BF16, kind="ExternalInput")
    mask_d = nc.dram_tensor("mask", [128, GRP_W], BF16, kind="ExternalInput")
    wg_d = nc.dram_tensor("wg", [128, 1], BF16, kind="ExternalInput")
    bgn_d = nc.dram_tensor("bgn", [1, 1], F32, kind="ExternalInput")
    out_d = nc.dram_tensor("out", [PPC, 128, NB * 64], F32, kind="ExternalOutput")

    with tile.TileContext(nc) as tc, ExitStack() as ctx:
        consts = ctx.enter_context(tc.tile_pool(name="consts", bufs=1))
        pairp = ctx.enter_context(tc.tile_pool(name="pairp", bufs=3))
        smalls = ctx.enter_context(tc.tile_pool(name="smalls", bufs=2))
        ppool = ctx.enter_context(tc.tile_pool(name="ppool", bufs=4))
        zpool = ctx.enter_context(tc.tile_pool(name="zpool", bufs=4))
        ps_s = ctx.enter_context(tc.tile_pool(name="ps_s", bufs=2, space="PSUM"))
        ps_pv = ctx.enter_context(tc.tile_pool(name="ps_pv", bufs=3, space="PSUM"))
        ps_g = ctx.enter_context(tc.tile_pool(name="ps_g", bufs=1, space="PSUM"))

        mask_sb = consts.tile([128, GRP_W], BF16, tag="mask")
        nc.sync.dma_start(out=mask_sb, in_=mask_d[:, :])
        wg_sb = consts.tile([128, 1], BF16, tag="wg")
        nc.sync.dma_start(out=wg_sb, in_=wg_d[:, :])
        bgn_sb = consts.tile([1, 1], F32, tag="bgn")
        nc.sync.dma_start(out=bgn_sb, in_=bgn_d[:, :])
        ones_sb = consts.tile([1, 128], F32, tag="ones")
        nc.vector.memset(ones_sb, 1.0)

        dis = DISABLE

        def emit_pair(p, qt2, kt2):
            b0 = (p % 2) * 64
            qt = qt2[b0 : b0 + 64, :]
            kt = kt2[b0 : b0 + 64, :]
            vg = pairp.tile([128, VS_W + NB * 64], BF16, tag="vg")
            if "dmain" not in dis:
                nc.sync.dma_start(out=vg, in_=vg_d[p])
            vs3 = vg[:, 0:VS_W].rearrange("p (c w) -> p c w", w=65)
            gl = vg[:, VS_W : VS_W + NB * 64]
            outp = pairp.tile([128, NB * 64], F32, tag="outp")

            if "gate" not in dis:
                # ---- gate: g = sigmoid(mean_s(q) . w + b), via exp only so
                # the ACT engine never has to swap activation tables ----
                g_ps = ps_g.tile([1, 512], F32, tag="gps")
                for t in range(4):
                    nc.tensor.matmul(
                        g_ps,
                        lhsT=wg_sb[b0 : b0 + 64, :],
                        rhs=qt[:, 32 + t * 512 : 32 + (t + 1) * 512],
                        start=(t == 0),
                        stop=(t == 3),
                    )
                # scl2 cols: 0 = sum(q.w), 3 = g, 4 = 1-g, 5 = 1/g = 1+exp(-x)
                scl2 = smalls.tile([1, 6], F32, tag="scl2")
                nc.vector.reduce_sum(scl2[:, 0:1], g_ps, axis=mybir.AxisListType.X)
                nc.scalar.activation(
                    scl2[:, 5:6],
                    scl2[:, 0:1],
                    mybir.ActivationFunctionType.Exp,
                    bias=bgn_sb[0:1, 0:1],
                    scale=-1.0 / S,
                )
                nc.vector.tensor_scalar(
                    scl2[:, 5:6], scl2[:, 5:6], 1.0, None, op0=mybir.AluOpType.add
                )
                nc.vector.reciprocal(scl2[:, 3:4], scl2[:, 5:6])
                nc.vector.tensor_scalar(
                    scl2[:, 4:5],
                    scl2[:, 3:4],
                    -1.0,
                    1.0,
                    op0=mybir.AluOpType.mult,
                    op1=mybir.AluOpType.add,
                )
                # broadcast (1-g, 1/g) across 128 partitions via ones matmul
                bc_ps = ps_g.tile([128, 2], F32, tag="gps")
                nc.tensor.matmul(
                    bc_ps, lhsT=ones_sb, rhs=scl2[:, 4:6], start=True, stop=True
                )
                bc = smalls.tile([128, 2], F32, tag="bc")
                nc.vector.tensor_copy(bc, bc_ps)
                # scale the appended V column (1 for valid keys) by 1/g so the
                # Z-column of pv comes out as Z/g and the tail scale is g/Z
                nc.vector.tensor_scalar_mul(
                    vs3[:, :, 64:65], vs3[:, :, 64:65], bc[:, 1:2]
                )
                # pre-scale the global path by (1-g); the per-block tail adds it
                nc.vector.tensor_scalar_mul(gl, gl, bc[:, 0:1])

            # ---- banded attention, 17 key chunks in groups of 6 ----
            pv_tiles = [None] * (NB // 2)

            def pv_slice(b, accum):
                t = pv_tiles[b // 2]
                lo = (b % 2) * 65
                return t[96:128, lo : lo + 65] if accum else t[:, lo : lo + 65]

            def _emit_tail(t):
                # blocks 2t and 2t+1 are fully accumulated in pv_tiles[t]:
                # out_blk = pv[:,0:64] * (g/Z) + (1-g)*global_blk.
                # Alternate engines per block to balance load: DVE gets the
                # fused scalar_tensor_tensor, ACT+GpSimd split the others.
                pvt = pv_tiles[t]
                pv3 = pvt.rearrange("p (b w) -> p b w", w=65)
                zr2 = zpool.tile([128, 2], F32, tag="zr")
                nc.vector.reciprocal(zr2, pv3[:, :, 64])
                for j in range(2):
                    b = 2 * t + j
                    if j % 2 == 0:
                        nc.vector.scalar_tensor_tensor(
                            outp[:, b * 64 : (b + 1) * 64],
                            pvt[:, j * 65 : j * 65 + 64],
                            zr2[:, j : j + 1],
                            gl[:, b * 64 : (b + 1) * 64],
                            op0=mybir.AluOpType.mult,
                            op1=mybir.AluOpType.add,
                        )
                    else:
                        tmp = zpool.tile([128, 64], F32, tag="tmp", name="tmp")
                        nc.scalar.activation(
                            tmp,
                            pvt[:, j * 65 : j * 65 + 64],
                            mybir.ActivationFunctionType.Copy,
                            bias=0.0,
                            scale=zr2[:, j : j + 1],
                        )
                        nc.gpsimd.tensor_add(
                            outp[:, b * 64 : (b + 1) * 64],
                            tmp,
                            gl[:, b * 64 : (b + 1) * 64],
                        )

            def emit_pv_phase(pT, chunks):
                for l, c in enumerate(chunks):
                    off = _goff(l)
                    # the accumulate-MM must be emitted BEFORE the start-MM:
                    # start=True clears has_written for the whole psum bank,
                    # and blocks 2t/2t+1 share a bank in pv_tiles[t]
                    if c > 0 and "pv" not in dis and "pvaccum" not in dis:
                        nc.tensor.matmul(
                            pv_slice(c - 1, True),
                            lhsT=pT[:, off : off + 32],
                            rhs=vs3[:, c, :],
                            start=False,
                            stop=True,
                            skip_group_check=True,
                            tile_position=(0, 96),
                        )
                    if c >= 2 and c % 2 == 0 and "tail" not in dis:
                        # pv tile (c//2 - 1) got its last accumulation above
                        _emit_tail(c // 2 - 1)
                    if c < NB:
                        if c % 2 == 0:
                            pv_tiles[c // 2] = ps_pv.tile(
                                [128, 130], F32, tag="pv", name="pv"
                            )
                        if "pv" not in dis and "pvstart" not in dis:
                            nc.tensor.matmul(
                                pv_slice(c, False),
                                lhsT=pT[:, off + 32 : off + SPAN],
                                rhs=vs3[:, c, :],
                                start=True,
                                stop=False,
                                skip_group_check=True,
                            )

            prev = [None]
            for g0 in range(0, NCH, GRP):
                chunks = range(g0, min(g0 + GRP, NCH))
                st = ps_s.tile([128, GRP_W], F32, tag="st")
                if "scores" not in dis:
                    for l, c in enumerate(chunks):
                        off = _goff(l)
                        nc.tensor.matmul(
                            st[:, off : off + SPAN],
                            lhsT=kt[:, c * 128 : (c + 1) * 128],
                            rhs=qt[:, c * 128 : c * 128 + SPAN],
                            start=True,
                            stop=True,
                        )
                pT = ppool.tile([128, GRP_W], BF16, tag="pT")
                if "exp" not in dis:
                    # exp(scores / sqrt(D)): one ACT op per 6 chunks amortizes
                    # the ~350-cycle fixed ACTIVATE overhead
                    nc.scalar.activation(
                        pT, st, mybir.ActivationFunctionType.Exp, scale=1.0 / np.sqrt(D)
                    )
                if "mask" not in dis:
                    # band masking; DVE is ~3.5x faster than GpSimd here, so
                    # give it two of three groups and GpSimd the last
                    meng = nc.vector if (g0 // GRP) % 3 < 2 else nc.gpsimd
                    meng.tensor_mul(pT, pT, mask_sb)
                if SWPIPE:
                    if prev[0] is not None:
                        emit_pv_phase(*prev[0])
                    prev[0] = (pT, chunks)
                else:
                    emit_pv_phase(pT, chunks)
            if SWPIPE and prev[0] is not None:
                emit_pv_phase(*prev[0])

            if "tail" not in dis:
                nc.sync.dma_start(out=out_d[p], in_=outp)
            else:
                # bench-only path: gl is bf16, so use the casting SWDGE DMA
                nc.gpsimd.dma_start(out=out_d[p], in_=gl)

        def emit_all():
            for grp in range(PPC // 2):
                qk2 = pairp.tile([128, QT_W + KT_W], BF16, tag="qk")
                nc.sync.dma_start(out=qk2, in_=qk_d[grp])
                for sub in range(2):
                    emit_pair(grp * 2 + sub, qk2[:, 0:QT_W], qk2[:, QT_W:])

        if reps is None:
            emit_all()
        else:
            # benchmark variant: repeat the whole body in-NEFF so wall-clock
            # deltas between rep counts measure pure HW iteration time
            engs = [
                mybir.EngineType.PE,
                mybir.EngineType.Activation,
                mybir.EngineType.DVE,
                mybir.EngineType.Pool,
                mybir.EngineType.SP,
            ]
            with tc.For_i(0, reps, 1, hint_engines=engs):
                for _ in range(BODY_MULT):
                    emit_all()


def _get_nc(reps=None):
    key = ("nc", reps, DISABLE, BODY_MULT, SWPIPE)
    if key not in _CACHE:
        nc = bacc.Bacc("TRN2", target_bir_lowering=False)
        _build_program(nc, reps=reps)
        nc.compile()
        _CACHE[key] = nc
    return _CACHE[key]


def _band_mask():
    j = np.arange(128)[:, None]
    i = np.arange(SPAN)[None, :]
    band = ((j <= i) & (j >= i - 32)).astype(NP_BF16)
    m = np.zeros((128, GRP_W), NP_BF16)
    for l in range(GRP):
        off = _goff(l)
        m[:, off : off + SPAN] = band
    return m


def _prepare_in_maps(inputs):
    q = np.ascontiguousarray(np.asarray(inputs["q"], dtype=np.float32))
    k = np.ascontiguousarray(np.asarray(inputs["k"], dtype=np.float32))
    v = np.ascontiguousarray(np.asarray(inputs["v"], dtype=np.float32))
    g = np.ascontiguousarray(np.asarray(inputs["global_out"], dtype=np.float32))
    wg = np.asarray(inputs["w_gate"], dtype=np.float32).reshape(64, 1)
    wg = np.ascontiguousarray(np.concatenate([wg, wg], axis=0).astype(NP_BF16))  # [128,1]
    bgn = -np.asarray(inputs["b_gate"], dtype=np.float32).reshape(1, 1)

    qf = q.reshape(PAIRS, S, D)
    kf = k.reshape(PAIRS, S, D)
    vf = v.reshape(PAIRS, S, D)
    gf = g.reshape(PAIRS, S, D)

    # host-side layout marshalling (transpose/pad/shift only, no math);
    # qt/kt stack pair 2j on partitions 0:64 and pair 2j+1 on 64:128
    qk = np.zeros((PAIRS // 2, 128, QT_W + KT_W), NP_BF16)
    qk[:, 0:64, 32 : 32 + S] = qf[0::2].transpose(0, 2, 1)
    qk[:, 64:128, 32 : 32 + S] = qf[1::2].transpose(0, 2, 1)
    qk[:, 0:64, QT_W + 16 : QT_W + 16 + S] = kf[0::2].transpose(0, 2, 1)
    qk[:, 64:128, QT_W + 16 : QT_W + 16 + S] = kf[1::2].transpose(0, 2, 1)

    vs = np.zeros((PAIRS, NCH * 128, 65), NP_BF16)
    vs[:, 16 : 16 + S, 0:64] = vf
    vs[:, 16 : 16 + S, 64] = 1.0
    vs = (
        vs.reshape(PAIRS, NCH, 128, 65)
        .transpose(0, 2, 1, 3)
        .reshape(PAIRS, 128, VS_W)
    )
    vs = np.ascontiguousarray(vs)

    gl = gf.reshape(PAIRS, NB, 128, 64).transpose(0, 2, 1, 3).reshape(PAIRS, 128, NB * 64)
    vg = np.ascontiguousarray(
        np.concatenate([vs, gl.astype(NP_BF16)], axis=2)
    )
    mask = _band_mask()

    in_maps = []
    for core in range(NCORES):
        lo, hi = core * PPC, (core + 1) * PPC
        glo, ghi = core * (PPC // 2), (core + 1) * (PPC // 2)
        in_maps.append(
            {
                "qk": np.ascontiguousarray(qk[glo:ghi]),
                "vg": vg[lo:hi],
                "mask": mask,
                "wg": wg,
                "bgn": bgn,
            }
        )
    return in_maps


def kernel(**inputs):
    global LAST_RESULT
    in_maps = _prepare_in_maps(inputs)
    nc = _get_nc()
    try:
        res = run_bass_kernel_spmd(
            nc, in_maps, core_ids=list(range(NCORES)), trace=TRACE
        )
    except ModuleNotFoundError:
        # NTFF profiling hook unavailable in this axon build
        res = run_bass_kernel_spmd(
            nc, in_maps, core_ids=list(range(NCORES)), trace=False
        )
    LAST_RESULT = res

    outs = np.stack([res.results[i]["out"] for i in range(NCORES)])  # [8,4,128,NB*64]
    out = (
        outs.reshape(PAIRS, 128, NB, 64)
        .transpose(0, 2, 1, 3)
        .reshape(B, H, S, D)
    )
    return np.ascontiguousarray(out)


def bench_hw_ns(inputs, reps_lo=16, reps_hi=2064, runs=5):
    """Estimate per-invocation HW time via in-NEFF repetition.

    Runs the same program with the body looped reps_lo and reps_hi times;
    the wall-clock delta divided by the rep delta isolates on-device time
    from compile/shipping/dispatch overhead.
    """
    import time

    in_maps = _prepare_in_maps(inputs)

    def run_variant(reps):
        nc = _get_nc(reps=reps)
        times = []
        for r in range(runs + 1):
            t0 = time.time()
            run_bass_kernel_spmd(nc, in_maps, core_ids=list(range(NCORES)))
            t1 = time.time()
            if r > 0:  # first run includes NEFF compile
                times.append(t1 - t0)
        return min(times)

    t_lo = run_variant(reps_lo)
    t_hi = run_variant(reps_hi)
    per_iter_ns = (t_hi - t_lo) / (reps_hi - reps_lo) * 1e9
    return per_iter_ns, t_lo, t_hi


if __name__ == "__main__":
    rng = np.random.default_rng(0)
    ins = {
        "q": rng.standard_normal((B, H, S, D), dtype=np.float32),
        "k": rng.standard_normal((B, H, S, D), dtype=np.float32),
        "v": rng.standard_normal((B, H, S, D), dtype=np.float32),
        "global_out": rng.standard_normal((B, H, S, D), dtype=np.float32),
        "buckets": rng.integers(0, 64, size=(B, S)),
        "w_gate": rng.standard_normal(64, dtype=np.float32) / 8.0,
        "b_gate": np.zeros(1, np.float32),
    }
    out = kernel(**ins)
    print("out", out.shape, out.dtype, float(np.abs(out).max()))



# revision 8
# speedup vs baseline: 1.8257x; 1.8257x over previous
"""Trainium2 Bass kernel for DynamicLocalGlobalRouter.

Reference computation (B=2, H=16, S=2048, D=64, radius=16):
  local_out = sliding-window softmax attention (window 33) per (b,h)
  gate      = sigmoid(mean_s(q) @ w_gate + b_gate)      per (b,h)
  out       = gate * local_out + (1-gate) * global_out

Sharding: B*H = 32 (b,h) pairs -> 4 pairs per core across 8 cores.

Device algorithm (per pair), key-stationary banded attention:
  - Host supplies q/k transposed+padded to [64, W] (bf16) so the
    contraction dim (d=64) is on partitions; two pairs stack on the
    partition axis so DMAs use all 16 ports.
  - For each key chunk c (17 chunks of 128 keys, shifted by -16):
      scores_T[k, q] = K_T_chunk.T @ Q_T_span       (PE, psum bank slots)
      scores_T += negband                           (PE identity-matmul
                                                     accumulate: adds -300
                                                     off-band, so no separate
                                                     mask multiply is needed)
      P_T = exp(scores_T / 8)                       (ACT, 6 chunks per op;
                                                     off-band -> exp(-37) ~ 0)
      pv[block] += P_T_cols.T @ [V | 1]             (PE; extra ones column
                                                     accumulates Z per query)
  - pv for a whole pair lives in 3 psum banks (7+7+2 blocks); per bank:
      zr = 1/Z (DVE reciprocal, strided), zrg = g*zr (Pool),
      out_blk = pv*zrg + (1-g)*global_blk  (fused DVE stt, a few via
      ACT-copy+Pool-add to balance engines)
  - gate: per-group column-sum of q (DVE reduce), 1-col fp32 matmul with
    w_gate, exp/sigmoid on tiny tiles, gpsimd partition_broadcast. The
    gate only feeds the tail (zrg / global prescale), never the pv path.
Output is bf16 (upcast on host); all softmax math stays fp32/bf16.
"""

import os
import sys
from contextlib import ExitStack

import numpy as np

sys.path.insert(0, "/opt/trn_rl_repo")

import concourse.bacc as bacc  # noqa: E402
import concourse.tile as tile  # noqa: E402
from concourse import mybir  # noqa: E402
from concourse.bass_utils import run_bass_kernel_spmd  # noqa: E402

B, H, S, D = 2, 16, 2048, 64
RADIUS = 16
NCORES = 8
PAIRS = B * H            # 32
PPC = PAIRS // NCORES    # 4 pairs per core
NB = S // 128            # 16 query blocks
NCH = NB + 1             # 17 key chunks (chunk c covers keys [c*128-16, c*128+112))
SPAN = 160               # query span per key chunk
QT_W = 32 + S            # 2080 padded Q_T width (col i <-> query i-32)
KT_W = 16 + S + 112      # 2176 padded K_T width (col j <-> key j-16)
VS_W = NCH * 65          # V chunks with appended ones column
GL_W = NB * 64           # 1024
GRP = 6                  # score chunks batched per psum group tile
GRP_W = 1024             # group tile width: 2 psum banks, 3 x 160 slots per bank
NBIAS = -300.0           # off-band score bias (exp(-300/8) ~ 5e-17)
PVB = ((0, 7), (7, 14), (14, 16))   # pv psum bank -> block range

F32 = mybir.dt.float32
BF16 = mybir.dt.bfloat16

import ml_dtypes  # noqa: E402

NP_BF16 = ml_dtypes.bfloat16

# set by test harness to capture an NTFF profile
TRACE = bool(int(os.environ.get("KERNEL_TRACE", "0")))
LAST_RESULT = None

_CACHE = {}

# bisection knobs for benchmarking: subset of
# {"scores", "exp", "pv", "tail", "dmain", "gate"}
DISABLE = frozenset()
# bench-only: how many copies of the body to emit per loop iteration
BODY_MULT = 1
# emit group g-1's PV matmuls after group g's scores (software pipelining)
SWPIPE = True
# blocks whose tail blend runs as ACT-copy + Pool-add instead of DVE stt
ACT_BLOCKS = frozenset({2, 9, 15})


def _goff(l):
    """free-dim offset of chunk-slot l (0..5) inside a group tile; slots
    avoid straddling the 2KB psum bank boundary (3 x 160 <= 512 per bank)"""
    return (l // 3) * 512 + (l % 3) * 160


def _build_program(nc, reps=None):
    # qt/kt stack two pairs on the partition axis (pair 2j on partitions
    # 0:64, pair 2j+1 on 64:128) so their DMAs use all 16 ports.
    qk_d = nc.dram_tensor("qk", [PPC // 2, 128, QT_W + KT_W], BF16, kind="ExternalInput")
    vs_d = nc.dram_tensor("vs", [PPC, 128, VS_W], BF16, kind="ExternalInput")
    gl_d = nc.dram_tensor("gl", [PPC, 128, GL_W], BF16, kind="ExternalInput")
    nb_d = nc.dram_tensor("nb", [128, GRP_W], BF16, kind="ExternalInput")
    id_d = nc.dram_tensor("ident", [128, 128], BF16, kind="ExternalInput")
    wg_d = nc.dram_tensor("wg", [128, 1], F32, kind="ExternalInput")
    bgn_d = nc.dram_tensor("bgn", [1, 1], F32, kind="ExternalInput")
    out_d = nc.dram_tensor("out", [PPC, 128, GL_W], BF16, kind="ExternalOutput")

    with tile.TileContext(nc) as tc, ExitStack() as ctx:
        consts = ctx.enter_context(tc.tile_pool(name="consts", bufs=1))
        qkpool = ctx.enter_context(tc.tile_pool(name="qkpool", bufs=2))
        vpool = ctx.enter_context(tc.tile_pool(name="vpool", bufs=3))
        glpool = ctx.enter_context(tc.tile_pool(name="glpool", bufs=3))
        ptpool = ctx.enter_context(tc.tile_pool(name="ptpool", bufs=3))
        outpool = ctx.enter_context(tc.tile_pool(name="outpool", bufs=3))
        smalls = ctx.enter_context(tc.tile_pool(name="smalls", bufs=3))
        zpool = ctx.enter_context(tc.tile_pool(name="zpool", bufs=6))
        ps_s = ctx.enter_context(tc.tile_pool(name="ps_s", bufs=2, space="PSUM"))
        ps_pv = ctx.enter_context(tc.tile_pool(name="ps_pv", bufs=3, space="PSUM"))
        ps_g = ctx.enter_context(tc.tile_pool(name="ps_g", bufs=1, space="PSUM"))

        nb_sb = consts.tile([128, GRP_W], BF16, tag="nb")
        nc.sync.dma_start(out=nb_sb, in_=nb_d[:, :])
        id_sb = consts.tile([128, 128], BF16, tag="ident")
        nc.sync.dma_start(out=id_sb, in_=id_d[:, :])
        wg_sb = consts.tile([128, 1], F32, tag="wg")
        nc.sync.dma_start(out=wg_sb, in_=wg_d[:, :])
        bgn_sb = consts.tile([1, 1], F32, tag="bgn")
        nc.sync.dma_start(out=bgn_sb, in_=bgn_d[:, :])

        dis = DISABLE

        def emit_pair(p, qt2, kt2, vs_t, gl_t, qsum):
            b0 = (p % 2) * 64
            qt = qt2[b0 : b0 + 64, :]
            kt = kt2[b0 : b0 + 64, :]
            vs3 = vs_t.rearrange("p (c w) -> p c w", w=65)
            outp = outpool.tile([128, GL_W], BF16, tag="outp")

            # ---- gate: g = sigmoid(mean_s(q) . w + b), via exp only so
            # the ACT engine never swaps activation tables. Never blocks
            # the pv path; consumed only by the per-bank tails. ----
            gb = smalls.tile([128, 2], F32, tag="gb")
            if "gate" not in dis:
                gmm = ps_g.tile([1, 1], F32, tag="gmm")
                nc.tensor.matmul(
                    gmm,
                    lhsT=wg_sb[b0 : b0 + 64, :],
                    rhs=qsum[b0 : b0 + 64, :],
                    start=True,
                    stop=True,
                )
                scl = smalls.tile([1, 4], F32, tag="scl")
                # scl cols: 0 = e = exp(-(x/S + b)), 1 = 1+e, 2 = g, 3 = 1-g
                nc.scalar.activation(
                    scl[:, 0:1],
                    gmm,
                    mybir.ActivationFunctionType.Exp,
                    bias=bgn_sb[0:1, 0:1],
                    scale=-1.0 / S,
                )
                nc.vector.tensor_scalar_add(scl[:, 1:2], scl[:, 0:1], 1.0)
                nc.vector.reciprocal(scl[:, 2:3], scl[:, 1:2])
                nc.vector.tensor_scalar(
                    scl[:, 3:4],
                    scl[:, 2:3],
                    -1.0,
                    1.0,
                    op0=mybir.AluOpType.mult,
                    op1=mybir.AluOpType.add,
                )
                nc.gpsimd.partition_broadcast(gb, scl[0:1, 2:4])
                # pre-scale the global path by (1-g); tails add it per block
                nc.vector.tensor_scalar_mul(gl_t, gl_t, gb[:, 1:2])

            # ---- banded attention, 17 key chunks in groups of 6 ----
            pv_banks = [None, None, None]

            def emit_bank_tail(bank):
                lo, hi = PVB[bank]
                nblk = hi - lo
                pv3 = pv_banks[bank][:, 0:455].rearrange("p (b w) -> p b w", w=65)
                zr = zpool.tile([128, nblk], F32, tag="zr", name="zr")
                nc.vector.reciprocal(zr, pv3[:, 0:nblk, 64])
                if "gate" not in dis:
                    zrg = zpool.tile([128, nblk], F32, tag="zrg", name="zrg")
                    nc.gpsimd.tensor_scalar_mul(zrg, zr, gb[:, 0:1])
                else:
                    zrg = zr
                for j in range(nblk):
                    b = lo + j
                    if b in ACT_BLOCKS:
                        tmp = zpool.tile([128, 64], F32, tag="tmp", name="tmp")
                        nc.scalar.activation(
                            tmp,
                            pv3[:, j, 0:64],
                            mybir.ActivationFunctionType.Copy,
                            bias=0.0,
                            scale=zrg[:, j : j + 1],
                        )
                        nc.gpsimd.tensor_add(
                            outp[:, b * 64 : (b + 1) * 64],
                            tmp,
                            gl_t[:, b * 64 : (b + 1) * 64],
                        )
                    else:
                        nc.vector.scalar_tensor_tensor(
                            outp[:, b * 64 : (b + 1) * 64],
                            pv3[:, j, 0:64],
                            zrg[:, j : j + 1],
                            gl_t[:, b * 64 : (b + 1) * 64],
                            op0=mybir.AluOpType.mult,
                            op1=mybir.AluOpType.add,
                        )

            def emit_pv_phase(pT, chunks):
                for l, c in enumerate(chunks):
                    off = _goff(l)
                    # the accumulate-MM must be emitted BEFORE the start-MM:
                    # start=True marks the whole psum bank pending-zero, and
                    # blocks share banks
                    if c > 0 and "pv" not in dis:
                        b = c - 1
                        bank, col = b // 7, (b % 7) * 65
                        nc.tensor.matmul(
                            pv_banks[bank][96:128, col : col + 65],
                            lhsT=pT[:, off : off + 32],
                            rhs=vs3[:, c, :],
                            start=False,
                            stop=(b in (6, 13, 15)),
                            skip_group_check=True,
                            tile_position=(0, 96),
                        )
                        if b in (6, 13, 15) and "tail" not in dis:
                            emit_bank_tail(b // 7)
                    if c < NB:
                        bank, col = c // 7, (c % 7) * 65
                        if c % 7 == 0:
                            # full-bank width so the 2048B row pitch keeps
                            # partition-sliced accum MMs inside one bank
                            pv_banks[bank] = ps_pv.tile(
                                [128, 512], F32, tag="pv", name="pv"
                            )
                        if "pv" not in dis:
                            nc.tensor.matmul(
                                pv_banks[bank][:, col : col + 65],
                                lhsT=pT[:, off + 32 : off + SPAN],
                                rhs=vs3[:, c, :],
                                start=True,
                                stop=False,
                                skip_group_check=True,
                            )

            pend = [None]
            for g0 in range(0, NCH, GRP):
                chunks = range(g0, min(g0 + GRP, NCH))
                st = ps_s.tile([128, GRP_W], F32, tag="st")
                hi = [0, 0]
                for l, c in enumerate(chunks):
                    hi[l // 3] = _goff(l) + (SPAN if c < NCH - 1 else 32)
                if "scores" not in dis:
                    for l, c in enumerate(chunks):
                        off = _goff(l)
                        n = SPAN if c < NCH - 1 else 32
                        nc.tensor.matmul(
                            st[:, off : off + n],
                            lhsT=kt[:, c * 128 : (c + 1) * 128],
                            rhs=qt[:, c * 128 : c * 128 + n],
                            start=(l % 3 == 0),
                            stop=False,
                            skip_group_check=True,
                        )
                    # add -300 off-band via identity matmul accumulate over
                    # exactly the written cols of each bank (pending-zero
                    # bytes must not mix with written ones in a single MM);
                    # replaces a separate mask multiply on DVE/Pool
                    for bk in range(2):
                        if hi[bk] == 0:
                            continue
                        nc.tensor.matmul(
                            st[:, bk * 512 : hi[bk]],
                            lhsT=id_sb,
                            rhs=nb_sb[:, bk * 512 : hi[bk]],
                            start=False,
                            stop=True,
                            skip_group_check=True,
                        )
                pT = ptpool.tile([128, GRP_W], BF16, tag="pT")
                if "exp" not in dis:
                    # exp(scores / sqrt(D)): one ACT op per psum bank (3
                    # chunks) amortizes the fixed ACTIVATE overhead; reads
                    # only written cols (never-written psum bytes are NaN
                    # sentinels in the simulator)
                    for bk in range(2):
                        if hi[bk] == 0:
                            continue
                        nc.scalar.activation(
                            pT[:, bk * 512 : hi[bk]],
                            st[:, bk * 512 : hi[bk]],
                            mybir.ActivationFunctionType.Exp,
                            scale=1.0 / np.sqrt(D),
                        )
                if SWPIPE:
                    if pend[0] is not None:
                        emit_pv_phase(*pend[0])
                    pend[0] = (pT, chunks)
                else:
                    emit_pv_phase(pT, chunks)
            if SWPIPE and pend[0] is not None:
                emit_pv_phase(*pend[0])

            if "tail" not in dis:
                nc.sync.dma_start(out=out_d[p], in_=outp)
            else:
                # bench-only path: outp never written; ship gl instead
                nc.sync.dma_start(out=out_d[p], in_=gl_t)

        def emit_all():
            # prefetch both qk groups up front; vs/gl one pair ahead
            qk_tiles = []
            for grp in range(PPC // 2):
                t = qkpool.tile([128, QT_W + KT_W], BF16, tag="qk", name="qk")
                nc.sync.dma_start(out=t, in_=qk_d[grp])
                qk_tiles.append(t)
            vs_tiles, gl_tiles = {}, {}

            def prefetch(p):
                if p >= PPC or p in vs_tiles:
                    return
                vs_tiles[p] = vpool.tile([128, VS_W], BF16, tag="vs", name="vs")
                gl_tiles[p] = glpool.tile([128, GL_W], BF16, tag="gl", name="gl")
                if "dmain" not in dis:
                    nc.sync.dma_start(out=vs_tiles[p], in_=vs_d[p])
                    nc.sync.dma_start(out=gl_tiles[p], in_=gl_d[p])

            prefetch(0)
            prefetch(1)
            qsums = {}
            for p in range(PPC):
                grp = p // 2
                prefetch(p + 1)
                qk2 = qk_tiles[grp]
                if p % 2 == 0:
                    # per-group column-sum of q over all queries (both pairs
                    # at once); feeds the two 1-col gate matmuls
                    qsums[grp] = smalls.tile([128, 1], F32, tag="qsum", name="qsum")
                    nc.vector.tensor_reduce(
                        qsums[grp],
                        qk2[:, 32:QT_W],
                        axis=mybir.AxisListType.X,
                        op=mybir.AluOpType.add,
                    )
                emit_pair(
                    p, qk2[:, 0:QT_W], qk2[:, QT_W:], vs_tiles.pop(p),
                    gl_tiles.pop(p), qsums[grp],
                )

        if reps is None:
            emit_all()
        else:
            # benchmark variant: repeat the whole body in-NEFF so wall-clock
            # deltas between rep counts measure pure HW iteration time
            engs = [
                mybir.EngineType.PE,
                mybir.EngineType.Activation,
                mybir.EngineType.DVE,
                mybir.EngineType.Pool,
                mybir.EngineType.SP,
            ]
            with tc.For_i(0, reps, 1, hint_engines=engs):
                for _ in range(BODY_MULT):
                    emit_all()


def _get_nc(reps=None):
    key = ("nc", reps, DISABLE, BODY_MULT, SWPIPE, ACT_BLOCKS)
    if key not in _CACHE:
        nc = bacc.Bacc("TRN2", target_bir_lowering=False)
        _build_program(nc, reps=reps)
        nc.compile()
        _CACHE[key] = nc
    return _CACHE[key]


def _negband():
    j = np.arange(128)[:, None]
    i = np.arange(SPAN)[None, :]
    band = np.where((j <= i) & (j >= i - 32), 0.0, NBIAS).astype(NP_BF16)
    m = np.full((128, GRP_W), NBIAS, NP_BF16)
    for l in range(GRP):
        off = _goff(l)
        m[:, off : off + SPAN] = band
    return m


def _prepare_in_maps(inputs):
    q = np.ascontiguousarray(np.asarray(inputs["q"], dtype=np.float32))
    k = np.ascontiguousarray(np.asarray(inputs["k"], dtype=np.float32))
    v = np.ascontiguousarray(np.asarray(inputs["v"], dtype=np.float32))
    g = np.ascontiguousarray(np.asarray(inputs["global_out"], dtype=np.float32))
    wg = np.asarray(inputs["w_gate"], dtype=np.float32).reshape(64, 1)
    wg = np.ascontiguousarray(np.concatenate([wg, wg], axis=0))  # [128,1] f32
    bgn = -np.asarray(inputs["b_gate"], dtype=np.float32).reshape(1, 1)

    qf = q.reshape(PAIRS, S, D)
    kf = k.reshape(PAIRS, S, D)
    vf = v.reshape(PAIRS, S, D)
    gf = g.reshape(PAIRS, S, D)

    # host-side layout marshalling (transpose/pad/shift only, no math);
    # qt/kt stack pair 2j on partitions 0:64 and pair 2j+1 on 64:128
    qk = np.zeros((PAIRS // 2, 128, QT_W + KT_W), NP_BF16)
    qk[:, 0:64, 32 : 32 + S] = qf[0::2].transpose(0, 2, 1)
    qk[:, 64:128, 32 : 32 + S] = qf[1::2].transpose(0, 2, 1)
    qk[:, 0:64, QT_W + 16 : QT_W + 16 + S] = kf[0::2].transpose(0, 2, 1)
    qk[:, 64:128, QT_W + 16 : QT_W + 16 + S] = kf[1::2].transpose(0, 2, 1)

    vs = np.zeros((PAIRS, NCH * 128, 65), NP_BF16)
    vs[:, 16 : 16 + S, 0:64] = vf
    vs[:, 16 : 16 + S, 64] = 1.0
    vs = (
        vs.reshape(PAIRS, NCH, 128, 65)
        .transpose(0, 2, 1, 3)
        .reshape(PAIRS, 128, VS_W)
    )
    vs = np.ascontiguousarray(vs)

    gl = np.ascontiguousarray(
        gf.reshape(PAIRS, NB, 128, 64)
        .transpose(0, 2, 1, 3)
        .reshape(PAIRS, 128, GL_W)
        .astype(NP_BF16)
    )
    nb = _negband()
    ident = np.eye(128, dtype=NP_BF16)

    in_maps = []
    for core in range(NCORES):
        lo, hi = core * PPC, (core + 1) * PPC
        glo, ghi = core * (PPC // 2), (core + 1) * (PPC // 2)
        in_maps.append(
            {
                "qk": np.ascontiguousarray(qk[glo:ghi]),
                "vs": vs[lo:hi],
                "gl": gl[lo:hi],
                "nb": nb,
                "ident": ident,
                "wg": wg,
                "bgn": bgn,
            }
        )
    return in_maps


def kernel(**inputs):
    global LAST_RESULT
    in_maps = _prepare_in_maps(inputs)
    nc = _get_nc()
    try:
        res = run_bass_kernel_spmd(
            nc, in_maps, core_ids=list(range(NCORES)), trace=TRACE
        )
    except ModuleNotFoundError:
        # NTFF profiling hook unavailable in this axon build
        res = run_bass_kernel_spmd(
            nc, in_maps, core_ids=list(range(NCORES)), trace=False
        )
    LAST_RESULT = res

    outs = np.stack([res.results[i]["out"] for i in range(NCORES)])  # [8,4,128,GL_W]
    out = (
        outs.astype(np.float32)
        .reshape(PAIRS, 128, NB, 64)
        .transpose(0, 2, 1, 3)
        .reshape(B, H, S, D)
    )
    return np.ascontiguousarray(out)


def bench_hw_ns(inputs, reps_lo=1024, reps_hi=17408, runs=6):
    """Estimate per-invocation HW time via in-NEFF repetition.

    Runs the same program with the body looped reps_lo and reps_hi times;
    the wall-clock delta divided by the rep delta isolates on-device time
    from compile/shipping/dispatch overhead.
    """
    import time

    in_maps = _prepare_in_maps(inputs)

    def run_variant(reps):
        nc = _get_nc(reps=reps)
        times = []
        for r in range(runs + 1):
            t0 = time.time()
            run_bass_kernel_spmd(nc, in_maps, core_ids=list(range(NCORES)))
            t1 = time.time()
            if r > 0:  # first run includes NEFF compile
                times.append(t1 - t0)
        return min(times)

    t_lo = run_variant(reps_lo)
    t_hi = run_variant(reps_hi)
    per_iter_ns = (t_hi - t_lo) / (reps_hi - reps_lo) * 1e9
    return per_iter_ns, t_lo, t_hi


if __name__ == "__main__":
    rng = np.random.default_rng(0)
    ins = {
        "q": rng.standard_normal((B, H, S, D), dtype=np.float32),
        "k": rng.standard_normal((B, H, S, D), dtype=np.float32),
        "v": rng.standard_normal((B, H, S, D), dtype=np.float32),
        "global_out": rng.standard_normal((B, H, S, D), dtype=np.float32),
        "buckets": rng.integers(0, 64, size=(B, S)),
        "w_gate": rng.standard_normal(64, dtype=np.float32) / 8.0,
        "b_gate": np.zeros(1, np.float32),
    }
    out = kernel(**ins)
    print("out", out.shape, out.dtype, float(np.abs(out).max()))
